# revision 14
# baseline (speedup 1.0000x reference)
"""Trainium2 Bass kernel for nn_BaseLinearSSM (chunked formulation).

y[b,t] = Re(C @ x_{t+1}) + D @ u[b,t] + bias,  x_{t+1} = A x_t + B u_t  (complex A,B,C)

Strategy (chunk length L=8, NK=T/L=256 chunks):
  Host (fp64): eigendecompose A = V diag(w) V^-1, Bt = V^-1 B, Ct = C V.
  Precompute:
    Pt_j = diag(w^(L-1-j)) Bt          [N,IN]  (chunk input aggregation)
    Qt_j = Ct diag(w^(j+1))            [OUT,N] (chunk boundary -> outputs)
    K_d  = Re(C A^d B), K_0 += D       [OUT,IN] real (within-chunk causal conv)
  Device (per core, batch-sharded 2 of 16; fp16 data, fp32 PSUM/scan state):
    phase 1: vt_k = sum_j Pt_j u_{kL+j}                    (matmuls, PSUM)
    phase 2: S_k = w^L S_{k-1} + vt_k  via modulate/scan/demodulate on the
             CHUNK axis only (T/L columns -> 1/8 the DVE work of a full scan);
             demod written with a one-chunk shift so S_shift[k] = beta_k =
             state at chunk start (col k=0 memset to 0 per batch element)
    phase 3: y_{kL+j} = Re(Qt_j beta_k) + sum_d K_d u_{kL+j-d}  (matmuls)
  Time is laid out (j, b, k) so every matmul has 512 contiguous columns.
  Phase 3 runs in two waves (j0..5, j6..7) with the boundary matmuls ordered
  m-outer, so the tensor engine only needs the last S tiles at the very end
  of wave A (phase-2 tail hidden behind conv + earlier-m matmuls).
  Input DMA is split over the two HWDGE rings (sync + scalar queues).
  Host shards u, permutes layouts, gathers y, adds bias.
"""

import sys

import numpy as np

if "/opt/trn_rl_repo" not in sys.path:
    sys.path.insert(0, "/opt/trn_rl_repo")

BATCH, T, IN, OUT, N = 16, 2048, 128, 128, 512
NCORES = 8
BLOCAL = BATCH // NCORES   # 2
L = 8                      # chunk length
NK = T // L                # 256 chunks per batch element
NKB = BLOCAL * NK          # 512 chunk-columns per core (b-major)
NT = N // 128              # 4 partition tiles over the state dim
COLS = BLOCAL * T          # 4096

# blob (fp16) layout:
#   u_jk [4096] | KT [1024] | per m: (PtT[m] [16*128] | ck2 | sk2 | rho2[m]) |
#   QtT [8192]
UW = L * NKB               # 4096
KW = L * 128               # 1024
PW = 2 * L * 128           # 2048 per m
TW = 2 * NKB               # 1024 per m (cos+sin)
RW = NKB                   # 512 per m (rho, col NK zeroed)
QW = L * 2 * NT * 128      # 8192
MW = PW + TW + RW          # 3584 per m
W16 = UW + KW + NT * MW + QW  # 27648

LAST_RESULT = None
_NC_CACHE = None


def _build_nc():
    from concourse import bass, mybir
    from concourse import tile

    f32 = mybir.dt.float32
    f16 = mybir.dt.float16
    op = mybir.AluOpType

    nc = bass.Bass("TRN2", target_bir_lowering=False, debug=False)

    blob = nc.dram_tensor("blob", [128, W16], f16, kind="ExternalInput")
    yout = nc.dram_tensor("y", [OUT, COLS], f32, kind="ExternalOutput")

    with tile.TileContext(nc) as tc:
        with (
            tc.tile_pool(name="const", bufs=1) as cpool,
            tc.tile_pool(name="vsb", bufs=2) as vpool,
            tc.tile_pool(name="tmp", bufs=2) as tpool,
            tc.tile_pool(name="gz", bufs=2) as gpool,
            tc.tile_pool(name="S", bufs=1) as spool,
            tc.tile_pool(name="ysb", bufs=4) as ypool_sb,
            tc.tile_pool(name="ps", bufs=1, space="PSUM") as pspool,
        ):
            b16 = cpool.tile([128, W16], f16)
            o = [0]

            def take(w):
                s = b16[:, o[0]:o[0] + w]
                o[0] += w
                return s

            u_jk = take(UW)
            ktT = [take(128) for _ in range(L)]
            ptT = [[[None] * L for _ in range(2)] for _ in range(NT)]
            ck2 = [None] * NT
            sk2 = [None] * NT
            rho2 = [None] * NT
            for m in range(NT):
                for ri in range(2):
                    for j in range(L):
                        ptT[m][ri][j] = take(128)
                ck2[m] = take(NKB)
                sk2[m] = take(NKB)
                rho2[m] = take(NKB)
            qtT = [[[None] * NT for _ in range(2)] for _ in range(L)]
            for j in range(L):
                for ri in range(2):
                    for m in range(NT):
                        qtT[j][ri][m] = take(128)
            assert o[0] == W16

            # DMA split over the two HWDGE rings:
            #   sync:   [u | K]  then [Qt]
            #   scalar: [Pt_m | tab_m | rho_m]  x 4
            a = UW + KW
            nc.sync.dma_start(b16[:, 0:a], blob[:, 0:a])
            nc.sync.dma_start(b16[:, W16 - QW:W16], blob[:, W16 - QW:W16])
            for m in range(NT):
                lo, hi = a + m * MW, a + (m + 1) * MW
                nc.scalar.dma_start(b16[:, lo:hi], blob[:, lo:hi])

            # PE warm-up: ~12 dependency-free matmuls on scratch run during
            # the DMA head, flipping the HAM clock gate to 8/8 (2.4 GHz)
            # before phase 1 issues. Output is discarded.
            wsc = cpool.tile([128, NKB], f16)
            nc.vector.memset(wsc[:], 0.0)
            for wi in range(12):
                wp = pspool.tile([128, NKB], f32, tag="vt0", bufs=1,
                                 name=f"warm{wi}")
                nc.tensor.matmul(wp[:], wsc[:, :128], wsc[:],
                                 start=True, stop=True)

            Sr_t = [None] * NT
            Si_t = [None] * NT
            for m in range(NT):
                # phase 1: vt = sum_j Pt_j u_j  (complex, PSUM)
                v_sb = [None, None]
                for ri in range(2):
                    vt = pspool.tile([128, NKB], f32, tag=f"vt{ri}", bufs=1,
                                     name=f"vt{ri}")
                    for j in range(L):
                        nc.tensor.matmul(
                            vt[:], ptT[m][ri][j], u_jk[:, j * NKB:(j + 1) * NKB],
                            start=(j == 0), stop=(j == L - 1),
                        )
                    v_sb[ri] = vpool.tile([128, NKB], f16, tag=f"v{ri}",
                                          name=f"v{ri}")
                    nc.scalar.copy(v_sb[ri][:], vt[:])
                vr, vi = v_sb
                # phase 2: modulate  g = e^{-i phi (k+1)} vt
                # (DVE: real part; GpSimd: imag part)
                t1 = tpool.tile([128, NKB], f16, tag="t1")
                t2 = tpool.tile([128, NKB], f16, tag="t2")
                nc.vector.tensor_tensor(t1[:], ck2[m], vr[:], op=op.mult)
                nc.vector.tensor_tensor(t2[:], sk2[m], vi[:], op=op.mult)
                gr = gpool.tile([128, NKB], f16, tag="gr")
                nc.vector.tensor_tensor(gr[:], t1[:], t2[:], op=op.add)
                t3 = tpool.tile([128, NKB], f16, tag="t3")
                t4 = tpool.tile([128, NKB], f16, tag="t4")
                nc.gpsimd.tensor_tensor(t3[:], ck2[m], vi[:], op=op.mult)
                nc.gpsimd.tensor_tensor(t4[:], sk2[m], vr[:], op=op.mult)
                gi = gpool.tile([128, NKB], f16, tag="gi")
                nc.gpsimd.tensor_tensor(gi[:], t3[:], t4[:], op=op.subtract)
                # scan along k; rho2 has col NK zeroed to reset state at the
                # second batch element (fp32 state, fp16 IO)
                zr = gpool.tile([128, NKB], f16, tag="zr")
                zi = gpool.tile([128, NKB], f16, tag="zi")
                nc.vector.tensor_tensor_scan(
                    zr[:], rho2[m], gr[:], 0.0, op0=op.mult, op1=op.add
                )
                nc.vector.tensor_tensor_scan(
                    zi[:], rho2[m], gi[:], 0.0, op0=op.mult, op1=op.add
                )
                # demodulate S = e^{i phi (k+1)} z, written shifted one chunk:
                # S_shift[:, b*NK + k] = S_{k-1} (= beta_k), col k=0 zeroed
                t5 = tpool.tile([128, NKB], f16, tag="t5")
                t6 = tpool.tile([128, NKB], f16, tag="t6")
                nc.vector.tensor_tensor(t5[:], ck2[m], zr[:], op=op.mult)
                nc.vector.tensor_tensor(t6[:], sk2[m], zi[:], op=op.mult)
                t7 = tpool.tile([128, NKB], f16, tag="t7")
                t8 = tpool.tile([128, NKB], f16, tag="t8")
                nc.vector.tensor_tensor(t7[:], sk2[m], zr[:], op=op.mult)
                nc.vector.tensor_tensor(t8[:], ck2[m], zi[:], op=op.mult)
                Sr = spool.tile([128, NKB], f16, tag=f"Sr{m}")
                Si = spool.tile([128, NKB], f16, tag=f"Si{m}")
                nc.vector.memset(Sr[:, 0:1], 0.0)
                nc.vector.memset(Sr[:, NK:NK + 1], 0.0)
                nc.gpsimd.memset(Si[:, 0:1], 0.0)
                nc.gpsimd.memset(Si[:, NK:NK + 1], 0.0)
                for b in range(BLOCAL):
                    a0 = b * NK
                    nc.vector.tensor_tensor(
                        Sr[:, a0 + 1:a0 + NK], t5[:, a0:a0 + NK - 1],
                        t6[:, a0:a0 + NK - 1], op=op.subtract,
                    )
                    nc.gpsimd.tensor_tensor(
                        Si[:, a0 + 1:a0 + NK], t7[:, a0:a0 + NK - 1],
                        t8[:, a0:a0 + NK - 1], op=op.add,
                    )
                Sr_t[m], Si_t[m] = Sr, Si

            # phase 3, two waves (j0..5, j6..7): conv first (only needs u/K),
            # then boundary matmuls m-outer so S(m) is consumed in completion
            # order; stop on the last accumulation (m=3, ri=1).
            for js in (range(0, 6), range(6, L)):
                yps = {}
                for j in js:
                    yps[j] = pspool.tile([128, NKB], f32, tag="y", bufs=6,
                                         name=f"y{j}")
                    for d in range(j + 1):
                        nc.tensor.matmul(
                            yps[j][:], ktT[d],
                            u_jk[:, (j - d) * NKB:(j - d + 1) * NKB],
                            start=(d == 0), stop=False, skip_group_check=True,
                        )
                for m in range(NT):
                    for j in js:
                        nc.tensor.matmul(
                            yps[j][:], qtT[j][0][m], Sr_t[m][:], start=False,
                            stop=False, skip_group_check=True,
                        )
                        nc.tensor.matmul(
                            yps[j][:], qtT[j][1][m], Si_t[m][:], start=False,
                            stop=(m == NT - 1), skip_group_check=True,
                        )
                for j in js:
                    ysb = ypool_sb.tile([128, NKB], f32, tag="ysb")
                    nc.scalar.copy(ysb[:], yps[j][:])
                    nc.gpsimd.dma_start(yout[:, j * NKB:(j + 1) * NKB], ysb[:])

    _legalize_multi_waits(nc)
    return nc


def _legalize_multi_waits(nc):
    """This walrus build accepts a single sync wait per instruction; split
    any multi-wait instruction into same-engine single-wait NoOps + the
    original carrying the last wait (program order chains them)."""
    import bass_rust
    from concourse import mybir

    uid = [0]
    for fn in nc.m.functions:
        for bb in fn.blocks:
            insts = bb.instructions
            new = []
            changed = False
            for inst in insts:
                si = inst.sync_info
                if si is not None and len(si.on_wait) > 1:
                    waits = list(si.on_wait)
                    for w in waits[:-1]:
                        uid[0] += 1
                        new.append(mybir.InstNoOp(
                            name=f"mwsplit-{uid[0]}",
                            engine=inst.engine,
                            ins=[], outs=[],
                            sync_info=bass_rust.SyncInfo(on_wait=[w], on_update=[]),
                        ))
                    inst.sync_info = bass_rust.SyncInfo(
                        on_wait=[waits[-1]], on_update=list(si.on_update)
                    )
                    changed = True
                new.append(inst)
            if changed:
                bb.instructions = new


def _host_prep(A_re, A_im, B_re, B_im, C_re, C_im, D_w):
    """fp64 eigendecomposition + chunked-formulation weight/table layouts.
    Returns shared fp16 tail of the blob: [128, W16 - UW]."""
    A = A_re.astype(np.float64) + 1j * A_im.astype(np.float64)
    w, V = np.linalg.eig(A)
    Vinv = np.linalg.inv(V)
    Bt = Vinv @ (B_re.astype(np.float64) + 1j * B_im.astype(np.float64))
    Ct = (C_re.astype(np.float64) + 1j * C_im.astype(np.float64)) @ V

    Pt = np.stack([(w ** (L - 1 - j))[:, None] * Bt for j in range(L)])  # [L,N,IN]
    Qt = np.stack([Ct * (w ** (j + 1))[None, :] for j in range(L)])      # [L,OUT,N]
    K = np.empty((L, OUT, IN))
    Ad = np.eye(N, dtype=complex)
    Bc = B_re.astype(np.float64) + 1j * B_im.astype(np.float64)
    Cc = C_re.astype(np.float64) + 1j * C_im.astype(np.float64)
    for d in range(L):
        K[d] = (Cc @ Ad @ Bc).real
        Ad = A @ Ad
    K[0] += D_w.astype(np.float64)

    wL = w ** L
    rhoL = np.abs(wL)
    phi = np.angle(wL)
    kk = np.arange(NK)
    cosk = np.cos(np.outer(phi, kk + 1))  # [N, NK]
    sink = np.sin(np.outer(phi, kk + 1))

    parts = []
    for d in range(L):
        parts.append(np.ascontiguousarray(K[d].T))  # [IN, OUT]
    for m in range(NT):
        sl = slice(m * 128, (m + 1) * 128)
        for Pp in (Pt.real, Pt.imag):
            for j in range(L):
                parts.append(np.ascontiguousarray(Pp[j].T[:, sl]))  # [IN, 128]
        parts.append(np.tile(cosk[sl], (1, BLOCAL)))  # [128, NKB]
        parts.append(np.tile(sink[sl], (1, BLOCAL)))
        rb = np.broadcast_to(rhoL[sl][:, None], (128, NKB)).copy()
        rb[:, NK] = 0.0  # reset scan state at second batch element
        parts.append(rb)
    for j in range(L):
        for Qp in (Qt[j].real, -Qt[j].imag):
            QT = np.ascontiguousarray(Qp.T)  # [N, OUT]
            for m in range(NT):
                parts.append(QT[m * 128:(m + 1) * 128])
    shared = np.concatenate(parts, axis=1).astype(np.float16)
    assert shared.shape == (128, W16 - UW)
    return shared


def _ensure_axon_hooks():
    """Provide antenv.axon_hooks if the image lacks it (needed only for
    trace=True NTFF profiling; run path works without)."""
    import types
    try:
        from antenv import axon_hooks  # noqa: F401
        return
    except ImportError:
        pass
    try:
        import antenv
        mod = types.ModuleType("antenv.axon_hooks")
        _hook = [None]
        mod.set_axon_ntff_profile_hook = lambda h: _hook.__setitem__(0, h)
        mod.get_axon_ntff_profile_hook = lambda: _hook[0]
        sys.modules["antenv.axon_hooks"] = mod
        antenv.axon_hooks = mod
        if "/root/.axon_site" not in sys.path:
            sys.path.insert(0, "/root/.axon_site")
        from trn_agent_boot.trn_boot import _ntff_profile_via_ctypes
        h = _ntff_profile_via_ctypes("/opt/axon/libaxon_pjrt.so")
        if h is not None:
            mod.set_axon_ntff_profile_hook(h)
    except Exception:
        pass


def kernel(u, A_re, A_im, B_re, B_im, C_re, C_im, D_w, output_bias):
    global LAST_RESULT, _NC_CACHE
    from concourse import bass_utils

    _ensure_axon_hooks()

    u = np.asarray(u, dtype=np.float32)
    shared = _host_prep(
        np.asarray(A_re), np.asarray(A_im), np.asarray(B_re), np.asarray(B_im),
        np.asarray(C_re), np.asarray(C_im), np.asarray(D_w)
    )

    if _NC_CACHE is None:
        _NC_CACHE = _build_nc()
    nc = _NC_CACHE

    in_maps = []
    for c in range(NCORES):
        up = u[BLOCAL * c:BLOCAL * (c + 1)]           # [2, T, IN]
        uc = up.reshape(BLOCAL, NK, L, IN)            # t = k*L + j
        u_jk = np.ascontiguousarray(
            uc.transpose(3, 2, 0, 1).reshape(IN, L * NKB)
        ).astype(np.float16)                          # col = j*NKB + b*NK + k
        in_maps.append({"blob": np.concatenate([u_jk, shared], axis=1)})

    res = bass_utils.run_bass_kernel_spmd(nc, in_maps, core_ids=list(range(NCORES)))
    LAST_RESULT = res

    y = np.empty((BATCH, T, OUT), dtype=np.float32)
    for c in range(NCORES):
        yd = res.results[c]["y"]                      # [OUT, L*NKB]
        y[BLOCAL * c:BLOCAL * (c + 1)] = (
            yd.reshape(OUT, L, BLOCAL, NK).transpose(2, 3, 1, 0)
            .reshape(BLOCAL, T, OUT)
        )
    y += np.asarray(output_bias, dtype=np.float32)
    return y


# revision 15
# speedup vs baseline: 1.0766x; 1.0766x over previous
"""Trainium2 Bass kernel for nn_BaseLinearSSM (chunked formulation).

y[b,t] = Re(C @ x_{t+1}) + D @ u[b,t] + bias,  x_{t+1} = A x_t + B u_t  (complex A,B,C)

Strategy (chunk length L=8, NK=T/L=256 chunks):
  Host (fp64): eigendecompose A = V diag(w) V^-1, Bt = V^-1 B, Ct = C V.
  Precompute:
    Pt_j = diag(w^(L-1-j)) Bt          [N,IN]  (chunk input aggregation)
    Qt_j = Ct diag(w^(j+1))            [OUT,N] (chunk boundary -> outputs)
    K_d  = Re(C A^d B), K_0 += D       [OUT,IN] real (within-chunk causal conv)
  Device (per core, batch-sharded 2 of 16; fp16 data, fp32 PSUM/scan state):
    phase 1: vt_k = sum_j Pt_j u_{kL+j}                    (matmuls, PSUM)
    phase 2: S_k = w^L S_{k-1} + vt_k  via modulate/scan/demodulate on the
             CHUNK axis only (T/L columns -> 1/8 the DVE work of a full scan);
             demod written with a one-chunk shift so S_shift[k] = beta_k =
             state at chunk start (col k=0 memset to 0 per batch element)
    phase 3: y_{kL+j} = Re(Qt_j beta_k) + sum_d K_d u_{kL+j-d}  (matmuls)
  Time is laid out (j, b, k) so every matmul has 512 contiguous columns.
  Phase 3 runs in two waves (j0..5, j6..7) with the boundary matmuls ordered
  m-outer, so the tensor engine only needs the last S tiles at the very end
  of wave A (phase-2 tail hidden behind conv + earlier-m matmuls).
  Input DMA is split over the two HWDGE rings (sync + scalar queues).
  Host shards u, permutes layouts, gathers y, adds bias.
"""

import sys

import numpy as np

if "/opt/trn_rl_repo" not in sys.path:
    sys.path.insert(0, "/opt/trn_rl_repo")

BATCH, T, IN, OUT, N = 16, 2048, 128, 128, 512
NCORES = 8
BLOCAL = BATCH // NCORES   # 2
L = 8                      # chunk length
NK = T // L                # 256 chunks per batch element
NKB = BLOCAL * NK          # 512 chunk-columns per core (b-major)
NT = N // 128              # 4 partition tiles over the state dim
COLS = BLOCAL * T          # 4096

# blob (fp16) layout / DMA piece order:
#   sync queue:   u | Pt0 | Pt1 | Pt2 | Pt3
#   scalar queue: K | tr0 | tr1 | tr2 | tr3 | (deferred) Qt
# with tr_m = ck2 | sk2 | rho2.  Qt's dma_start is issued mid-phase-1 so its
# 2 MB does not steal HBM bandwidth from the phase-1-critical pieces.
UW = L * NKB               # 4096
KW = L * 128               # 1024
PW = 2 * L * 128           # 2048 per m
TW = 2 * NKB               # 1024 per m (cos+sin)
RW = NKB                   # 512 per m (rho, col NK zeroed)
QW = L * 2 * NT * 128      # 8192
TRW = TW + RW              # 1536 per m
W16 = UW + NT * PW + KW + NT * TRW + QW  # 27648

LAST_RESULT = None
_NC_CACHE = None


def _build_nc():
    from concourse import bass, mybir
    from concourse import tile

    f32 = mybir.dt.float32
    f16 = mybir.dt.float16
    op = mybir.AluOpType

    nc = bass.Bass("TRN2", target_bir_lowering=False, debug=False)

    blob = nc.dram_tensor("blob", [128, W16], f16, kind="ExternalInput")
    yout = nc.dram_tensor("y", [OUT, COLS], f32, kind="ExternalOutput")

    with tile.TileContext(nc) as tc:
        with (
            tc.tile_pool(name="const", bufs=1) as cpool,
            tc.tile_pool(name="vsb", bufs=2) as vpool,
            tc.tile_pool(name="tmp", bufs=2) as tpool,
            tc.tile_pool(name="gz", bufs=2) as gpool,
            tc.tile_pool(name="S", bufs=1) as spool,
            tc.tile_pool(name="ysb", bufs=4) as ypool_sb,
            tc.tile_pool(name="ps", bufs=1, space="PSUM") as pspool,
        ):
            b16 = cpool.tile([128, W16], f16)
            o = [0]

            def take(w):
                s = b16[:, o[0]:o[0] + w]
                o[0] += w
                return s

            u_jk = take(UW)
            ptT = [[[None] * L for _ in range(2)] for _ in range(NT)]
            for m in range(NT):
                for ri in range(2):
                    for j in range(L):
                        ptT[m][ri][j] = take(128)
            ktT = [take(128) for _ in range(L)]
            ck2 = [None] * NT
            sk2 = [None] * NT
            rho2 = [None] * NT
            for m in range(NT):
                ck2[m] = take(NKB)
                sk2[m] = take(NKB)
                rho2[m] = take(NKB)
            qtT = [[[None] * NT for _ in range(2)] for _ in range(L)]
            for j in range(L):
                for ri in range(2):
                    for m in range(NT):
                        qtT[j][ri][m] = take(128)
            assert o[0] == W16

            # sync queue: u, then each Pt piece (phase-1 critical path)
            nc.sync.dma_start(b16[:, 0:UW], blob[:, 0:UW])
            for m in range(NT):
                lo, hi = UW + m * PW, UW + (m + 1) * PW
                nc.sync.dma_start(b16[:, lo:hi], blob[:, lo:hi])
            # scalar queue: K, then per-m tables (phase-2 / conv)
            a = UW + NT * PW
            nc.scalar.dma_start(b16[:, a:a + KW], blob[:, a:a + KW])
            for m in range(NT):
                lo, hi = a + KW + m * TRW, a + KW + (m + 1) * TRW
                nc.scalar.dma_start(b16[:, lo:hi], blob[:, lo:hi])

            # PE warm-up: ~12 dependency-free matmuls on scratch run during
            # the DMA head, flipping the HAM clock gate to 8/8 (2.4 GHz)
            # before phase 1 issues. Output is discarded.
            wsc = cpool.tile([128, NKB], f16)
            nc.vector.memset(wsc[:], 0.0)
            wp = pspool.tile([128, NKB], f32, tag="vt0", bufs=1, name="warm")
            for wi in range(12):
                nc.tensor.matmul(wp[:], wsc[:, :128], wsc[:],
                                 start=(wi == 0), stop=(wi == 11))

            Sr_t = [None] * NT
            Si_t = [None] * NT
            for m in range(NT):
                # phase 1: vt = sum_j Pt_j u_j  (complex, PSUM)
                v_sb = [None, None]
                for ri in range(2):
                    vt = pspool.tile([128, NKB], f32, tag=f"vt{ri}", bufs=1,
                                     name=f"vt{ri}")
                    for j in range(L):
                        nc.tensor.matmul(
                            vt[:], ptT[m][ri][j], u_jk[:, j * NKB:(j + 1) * NKB],
                            start=(j == 0), stop=(j == L - 1),
                        )
                    v_sb[ri] = vpool.tile([128, NKB], f16, tag=f"v{ri}",
                                          name=f"v{ri}")
                    nc.scalar.copy(v_sb[ri][:], vt[:])
                if m == 1:
                    # deferred: Qt's 2 MB rides the scalar queue only after
                    # the phase-1-critical DMA pieces have landed
                    nc.scalar.dma_start(b16[:, W16 - QW:W16],
                                        blob[:, W16 - QW:W16])
                vr, vi = v_sb
                # phase 2: modulate  g = e^{-i phi (k+1)} vt
                # (DVE: real part; GpSimd: imag part)
                t1 = tpool.tile([128, NKB], f16, tag="t1")
                t2 = tpool.tile([128, NKB], f16, tag="t2")
                nc.vector.tensor_tensor(t1[:], ck2[m], vr[:], op=op.mult)
                nc.vector.tensor_tensor(t2[:], sk2[m], vi[:], op=op.mult)
                gr = gpool.tile([128, NKB], f16, tag="gr")
                nc.vector.tensor_tensor(gr[:], t1[:], t2[:], op=op.add)
                t3 = tpool.tile([128, NKB], f16, tag="t3")
                t4 = tpool.tile([128, NKB], f16, tag="t4")
                nc.gpsimd.tensor_tensor(t3[:], ck2[m], vi[:], op=op.mult)
                nc.gpsimd.tensor_tensor(t4[:], sk2[m], vr[:], op=op.mult)
                gi = gpool.tile([128, NKB], f16, tag="gi")
                nc.gpsimd.tensor_tensor(gi[:], t3[:], t4[:], op=op.subtract)
                # scan along k; rho2 has col NK zeroed to reset state at the
                # second batch element (fp32 state, fp16 IO)
                zr = gpool.tile([128, NKB], f16, tag="zr")
                zi = gpool.tile([128, NKB], f16, tag="zi")
                nc.vector.tensor_tensor_scan(
                    zr[:], rho2[m], gr[:], 0.0, op0=op.mult, op1=op.add
                )
                nc.vector.tensor_tensor_scan(
                    zi[:], rho2[m], gi[:], 0.0, op0=op.mult, op1=op.add
                )
                # demodulate S = e^{i phi (k+1)} z, written shifted one chunk:
                # S_shift[:, b*NK + k] = S_{k-1} (= beta_k), col k=0 zeroed
                t5 = tpool.tile([128, NKB], f16, tag="t5")
                t6 = tpool.tile([128, NKB], f16, tag="t6")
                nc.vector.tensor_tensor(t5[:], ck2[m], zr[:], op=op.mult)
                nc.vector.tensor_tensor(t6[:], sk2[m], zi[:], op=op.mult)
                t7 = tpool.tile([128, NKB], f16, tag="t7")
                t8 = tpool.tile([128, NKB], f16, tag="t8")
                nc.vector.tensor_tensor(t7[:], sk2[m], zr[:], op=op.mult)
                nc.vector.tensor_tensor(t8[:], ck2[m], zi[:], op=op.mult)
                Sr = spool.tile([128, NKB], f16, tag=f"Sr{m}")
                Si = spool.tile([128, NKB], f16, tag=f"Si{m}")
                nc.vector.memset(Sr[:, 0:1], 0.0)
                nc.vector.memset(Sr[:, NK:NK + 1], 0.0)
                nc.gpsimd.memset(Si[:, 0:1], 0.0)
                nc.gpsimd.memset(Si[:, NK:NK + 1], 0.0)
                for b in range(BLOCAL):
                    a0 = b * NK
                    nc.vector.tensor_tensor(
                        Sr[:, a0 + 1:a0 + NK], t5[:, a0:a0 + NK - 1],
                        t6[:, a0:a0 + NK - 1], op=op.subtract,
                    )
                    nc.gpsimd.tensor_tensor(
                        Si[:, a0 + 1:a0 + NK], t7[:, a0:a0 + NK - 1],
                        t8[:, a0:a0 + NK - 1], op=op.add,
                    )
                Sr_t[m], Si_t[m] = Sr, Si

            # phase 3, two waves (j0..5, j6..7): conv first (only needs u/K),
            # then boundary matmuls m-outer so S(m) is consumed in completion
            # order; stop on the last accumulation (m=3, ri=1).
            for js in (range(0, 6), range(6, L)):
                yps = {}
                for j in js:
                    yps[j] = pspool.tile([128, NKB], f32, tag="y", bufs=6,
                                         name=f"y{j}")
                    for d in range(j + 1):
                        nc.tensor.matmul(
                            yps[j][:], ktT[d],
                            u_jk[:, (j - d) * NKB:(j - d + 1) * NKB],
                            start=(d == 0), stop=False, skip_group_check=True,
                        )
                for m in range(NT):
                    for j in js:
                        nc.tensor.matmul(
                            yps[j][:], qtT[j][0][m], Sr_t[m][:], start=False,
                            stop=False, skip_group_check=True,
                        )
                        nc.tensor.matmul(
                            yps[j][:], qtT[j][1][m], Si_t[m][:], start=False,
                            stop=(m == NT - 1), skip_group_check=True,
                        )
                for j in js:
                    ysb = ypool_sb.tile([128, NKB], f32, tag="ysb")
                    nc.scalar.copy(ysb[:], yps[j][:])
                    nc.gpsimd.dma_start(yout[:, j * NKB:(j + 1) * NKB], ysb[:])

    _legalize_multi_waits(nc)
    return nc


def _legalize_multi_waits(nc):
    """This walrus build accepts a single sync wait per instruction; split
    any multi-wait instruction into same-engine single-wait NoOps + the
    original carrying the last wait (program order chains them)."""
    import bass_rust
    from concourse import mybir

    uid = [0]
    for fn in nc.m.functions:
        for bb in fn.blocks:
            insts = bb.instructions
            new = []
            changed = False
            for inst in insts:
                si = inst.sync_info
                if si is not None and len(si.on_wait) > 1:
                    waits = list(si.on_wait)
                    for w in waits[:-1]:
                        uid[0] += 1
                        new.append(mybir.InstNoOp(
                            name=f"mwsplit-{uid[0]}",
                            engine=inst.engine,
                            ins=[], outs=[],
                            sync_info=bass_rust.SyncInfo(on_wait=[w], on_update=[]),
                        ))
                    inst.sync_info = bass_rust.SyncInfo(
                        on_wait=[waits[-1]], on_update=list(si.on_update)
                    )
                    changed = True
                new.append(inst)
            if changed:
                bb.instructions = new


def _host_prep(A_re, A_im, B_re, B_im, C_re, C_im, D_w):
    """fp64 eigendecomposition + chunked-formulation weight/table layouts.
    Returns shared fp16 tail of the blob: [128, W16 - UW]."""
    A = A_re.astype(np.float64) + 1j * A_im.astype(np.float64)
    w, V = np.linalg.eig(A)
    Vinv = np.linalg.inv(V)
    Bt = Vinv @ (B_re.astype(np.float64) + 1j * B_im.astype(np.float64))
    Ct = (C_re.astype(np.float64) + 1j * C_im.astype(np.float64)) @ V

    Pt = np.stack([(w ** (L - 1 - j))[:, None] * Bt for j in range(L)])  # [L,N,IN]
    Qt = np.stack([Ct * (w ** (j + 1))[None, :] for j in range(L)])      # [L,OUT,N]
    K = np.empty((L, OUT, IN))
    Ad = np.eye(N, dtype=complex)
    Bc = B_re.astype(np.float64) + 1j * B_im.astype(np.float64)
    Cc = C_re.astype(np.float64) + 1j * C_im.astype(np.float64)
    for d in range(L):
        K[d] = (Cc @ Ad @ Bc).real
        Ad = A @ Ad
    K[0] += D_w.astype(np.float64)

    wL = w ** L
    rhoL = np.abs(wL)
    phi = np.angle(wL)
    kk = np.arange(NK)
    cosk = np.cos(np.outer(phi, kk + 1))  # [N, NK]
    sink = np.sin(np.outer(phi, kk + 1))

    parts = []
    for m in range(NT):
        sl = slice(m * 128, (m + 1) * 128)
        for Pp in (Pt.real, Pt.imag):
            for j in range(L):
                parts.append(np.ascontiguousarray(Pp[j].T[:, sl]))  # [IN, 128]
    for d in range(L):
        parts.append(np.ascontiguousarray(K[d].T))  # [IN, OUT]
    for m in range(NT):
        sl = slice(m * 128, (m + 1) * 128)
        parts.append(np.tile(cosk[sl], (1, BLOCAL)))  # [128, NKB]
        parts.append(np.tile(sink[sl], (1, BLOCAL)))
        rb = np.broadcast_to(rhoL[sl][:, None], (128, NKB)).copy()
        rb[:, NK] = 0.0  # reset scan state at second batch element
        parts.append(rb)
    for j in range(L):
        for Qp in (Qt[j].real, -Qt[j].imag):
            QT = np.ascontiguousarray(Qp.T)  # [N, OUT]
            for m in range(NT):
                parts.append(QT[m * 128:(m + 1) * 128])
    shared = np.concatenate(parts, axis=1).astype(np.float16)
    assert shared.shape == (128, W16 - UW)
    return shared


def _ensure_axon_hooks():
    """Provide antenv.axon_hooks if the image lacks it (needed only for
    trace=True NTFF profiling; run path works without)."""
    import types
    try:
        from antenv import axon_hooks  # noqa: F401
        return
    except ImportError:
        pass
    try:
        import antenv
        mod = types.ModuleType("antenv.axon_hooks")
        _hook = [None]
        mod.set_axon_ntff_profile_hook = lambda h: _hook.__setitem__(0, h)
        mod.get_axon_ntff_profile_hook = lambda: _hook[0]
        sys.modules["antenv.axon_hooks"] = mod
        antenv.axon_hooks = mod
        if "/root/.axon_site" not in sys.path:
            sys.path.insert(0, "/root/.axon_site")
        from trn_agent_boot.trn_boot import _ntff_profile_via_ctypes
        h = _ntff_profile_via_ctypes("/opt/axon/libaxon_pjrt.so")
        if h is not None:
            mod.set_axon_ntff_profile_hook(h)
    except Exception:
        pass


def kernel(u, A_re, A_im, B_re, B_im, C_re, C_im, D_w, output_bias):
    global LAST_RESULT, _NC_CACHE
    from concourse import bass_utils

    _ensure_axon_hooks()

    u = np.asarray(u, dtype=np.float32)
    shared = _host_prep(
        np.asarray(A_re), np.asarray(A_im), np.asarray(B_re), np.asarray(B_im),
        np.asarray(C_re), np.asarray(C_im), np.asarray(D_w)
    )

    if _NC_CACHE is None:
        _NC_CACHE = _build_nc()
    nc = _NC_CACHE

    in_maps = []
    for c in range(NCORES):
        up = u[BLOCAL * c:BLOCAL * (c + 1)]           # [2, T, IN]
        uc = up.reshape(BLOCAL, NK, L, IN)            # t = k*L + j
        u_jk = np.ascontiguousarray(
            uc.transpose(3, 2, 0, 1).reshape(IN, L * NKB)
        ).astype(np.float16)                          # col = j*NKB + b*NK + k
        in_maps.append({"blob": np.concatenate([u_jk, shared], axis=1)})

    res = bass_utils.run_bass_kernel_spmd(nc, in_maps, core_ids=list(range(NCORES)))
    LAST_RESULT = res

    y = np.empty((BATCH, T, OUT), dtype=np.float32)
    for c in range(NCORES):
        yd = res.results[c]["y"]                      # [OUT, L*NKB]
        y[BLOCAL * c:BLOCAL * (c + 1)] = (
            yd.reshape(OUT, L, BLOCAL, NK).transpose(2, 3, 1, 0)
            .reshape(BLOCAL, T, OUT)
        )
    y += np.asarray(output_bias, dtype=np.float32)
    return y
